# revision 11
# baseline (speedup 1.0000x reference)
"""Trainium2 Bass kernel for local-window multihead attention.

Problem: B=8, L=1024, C=1024, H=16 heads, head_dim=64, window_size=128
(positions attend to |i-j| <= 64). qkv in-projection + banded softmax
attention + out-projection.

Sharding: data-parallel - one batch element per NeuronCore (8 cores).

Per-core dataflow (bf16 matmuls, fp32 PSUM accumulation):
  xT (host-transposed bf16) --QK-proj--> qT/ktpad  [channel, seq] layout
                            --V-proj --> vpad      [seq, channel] layout
  attention is K-MAJOR: for each (head pair, key-chunk kc) compute
  St = K_chunk^T Q_window for both heads (row-group-packed K=64 matmuls),
  exp on ScalarE, multiplicative band mask on VectorE, then
  O^T += V_chunk^T @ P_t accumulated in PSUM across overlapping chunks,
  rowsums via indicator matmuls into a per-pair PSUM tile.  Per-pair
  softmax normalization (recip + K=2 replication matmul + fused
  multiply-evacuate) overlaps the next pair's attention.  QK-projection
  of pair hp+1 is interleaved into pair hp's stream to keep TensorE
  dense (HAM stays at 2.4 GHz).  out-proj from oT [c, l] -> y [l, c].
"""

import numpy as np
import ml_dtypes

import concourse.bass as bass
import concourse.mybir as mybir
import concourse.tile as tile
from concourse import bacc
from concourse.bass_utils import run_bass_kernel_spmd

BF16 = mybir.dt.bfloat16
F32 = mybir.dt.float32
F32R = mybir.dt.float32r

B, L, C, H, HD = 8, 1024, 1024, 16, 64
WIN = 128
NCORES = 8
NQT = L // 128
NCT = C // 128
NKC = NQT + 1           # key chunks in padded [-64, L+64) key space
SCALE = 1.0 / 8.0       # 1/sqrt(HD)

_CACHED = {}


def _build_nc():
    nc = bacc.Bacc(
        "TRN2", target_bir_lowering=False, debug=False, num_devices=NCORES
    )

    xT_d = nc.dram_tensor("xT", [C, L], BF16, kind="ExternalInput").ap()
    wqk_d = nc.dram_tensor("wqkT", [C, 3 * C], BF16, kind="ExternalInput").ap()
    wo_d = nc.dram_tensor("woT", [C, C], BF16, kind="ExternalInput").ap()
    bqk_d = nc.dram_tensor("bqk", [2 * C], F32, kind="ExternalInput").ap()
    bv_d = nc.dram_tensor("bv", [C], F32, kind="ExternalInput").ap()
    y_d = nc.dram_tensor("y", [L, C], F32, kind="ExternalOutput").ap()

    AF = mybir.ActivationFunctionType
    ALU = mybir.AluOpType

    with tile.TileContext(nc) as tc:
        with (
            tc.tile_pool(name="const", bufs=1) as const,
            tc.tile_pool(name="work", bufs=3) as work,
        ):
            # ---- persistent SBUF tensors ----
            wqk_s = const.tile([128, NCT, 3 * C], BF16, tag="wqk")  # [c_p, ct, o]
            wo_s = const.tile([128, NCT, C], BF16, tag="wo")
            xT_s = const.tile([128, NCT, L], BF16, tag="xT")        # [c_p, ct, l]
            qT_s = const.tile([128, NQT, L], BF16, tag="qT")        # [q-chan, ot, l]
            ktp_s = const.tile([128, NQT, L + 128], BF16, tag="ktp")  # k padded
            vp_s = const.tile([128, NKC, C], BF16, tag="vp")        # v rows +64
            oT_s = const.tile([128, NCT, L], BF16, tag="oT")        # [c_p, ct, l]
            bqk_s = const.tile([128, 2 * C // 128], F32, tag="bqk")
            bv_row = const.tile([1, C], F32, tag="bvrow")
            bvrep_s = const.tile([128, C], F32, tag="bvrep")
            ones1_s = const.tile([1, 128], F32, tag="ones1")
            zrow_s = const.tile([1, 512], BF16, tag="zrow")
            masks_s = const.tile([128, 3, 256], BF16, tag="masks")
            e2c_s = const.tile([128, 2, 2], BF16, tag="e2c")   # rowsum pick
            e2r_s = const.tile([2, 128], BF16, tag="e2r")       # rc replicate

            # ---- input DMAs, staged so phase 1 starts early ----
            nc.sync.dma_start(bqk_s[:], bqk_d.rearrange("(ot p) -> p ot", p=128))
            nc.sync.dma_start(bv_row[:], bv_d.rearrange("(p c) -> p c", p=1))
            nc.sync.dma_start(xT_s[:], xT_d.rearrange("(ct p) l -> p ct l", p=128))
            wqk_r = wqk_d.rearrange("(ct p) o -> p ct o", p=128)
            # q/k columns for early head pairs first, v-part last
            for o0 in (0, 1024, 512, 1536, 2048, 2560):
                nc.sync.dma_start(wqk_s[:, :, o0 : o0 + 512],
                                  wqk_r[:, :, o0 : o0 + 512])
            nc.sync.dma_start(wo_s[:], wo_d.rearrange("(ct p) o -> p ct o", p=128))

            # ---- constants ----
            nc.gpsimd.memset(ones1_s[:], 1.0)
            nc.gpsimd.memset(zrow_s[:], 0.0)

            # band mask (kc interior): valid iff 0 <= jq - p <= 128
            m_band = masks_s[:, 0, :]
            nc.gpsimd.memset(m_band, 1.0)
            nc.gpsimd.affine_select(
                m_band, m_band, compare_op=ALU.is_ge, fill=0.0,
                base=0, pattern=[[1, 256]], channel_multiplier=-1,
            )
            nc.gpsimd.affine_select(
                m_band, m_band, compare_op=ALU.is_ge, fill=0.0,
                base=128, pattern=[[-1, 256]], channel_multiplier=1,
            )
            # kc = 0 (cols [0,128)): valid iff p >= jq and p >= 64
            m_lo = masks_s[:, 1, 0:128]
            nc.gpsimd.memset(m_lo, 1.0)
            nc.gpsimd.affine_select(
                m_lo, m_lo, compare_op=ALU.is_ge, fill=0.0,
                base=0, pattern=[[-1, 128]], channel_multiplier=1,
            )
            nc.gpsimd.memset(masks_s[0:64, 1, 0:128], 0.0)
            # kc = NKC-1 (cols [0,128)): valid iff jq >= p and p < 64
            m_hi = masks_s[:, 2, 0:128]
            nc.gpsimd.memset(m_hi, 1.0)
            nc.gpsimd.affine_select(
                m_hi, m_hi, compare_op=ALU.is_ge, fill=0.0,
                base=0, pattern=[[1, 128]], channel_multiplier=-1,
            )
            nc.gpsimd.memset(masks_s[64:128, 2, 0:128], 0.0)

            # e2c[p, hh, j] = (j == hh); e2r[j, m] = (m // 64 == j)
            nc.gpsimd.memset(e2c_s[:], 1.0)
            nc.gpsimd.affine_select(
                e2c_s[:], e2c_s[:], compare_op=ALU.is_equal, fill=0.0,
                base=0, pattern=[[-1, 2], [1, 2]], channel_multiplier=0,
            )
            nc.gpsimd.memset(e2r_s[:], 1.0)
            nc.gpsimd.affine_select(
                e2r_s[:], e2r_s[:], compare_op=ALU.is_equal, fill=0.0,
                base=0, pattern=[[1, 2], [0, 64]], channel_multiplier=-1,
            )

            # zero padded edges of ktpad / vpad
            for ot in range(NQT):
                nc.gpsimd.memset(ktp_s[:, ot, 0:64], 0.0)
                nc.gpsimd.memset(ktp_s[:, ot, L + 64 : L + 128], 0.0)
            nc.gpsimd.memset(vp_s[0:64, 0, :], 0.0)
            nc.gpsimd.memset(vp_s[64:128, NKC - 1, :], 0.0)

            # ---- phases 1-3: projections + k-major attention ----
            with tc.tile_pool(name="psB", bufs=1, space="PSUM") as psB:

                def emit_qk_proj(ot, lt):
                    ps = psB.tile([128, 512], F32, tag="ps1", bufs=3,
                                  name=f"qkp{ot}_{lt}")
                    for ct in range(NCT):
                        nc.tensor.matmul(
                            ps[:],
                            lhsT=wqk_s[:, ct, ot * 128 : (ot + 1) * 128],
                            rhs=xT_s[:, ct, lt * 512 : (lt + 1) * 512],
                            start=(ct == 0), stop=(ct == NCT - 1),
                        )
                    if ot < NQT:
                        dest = qT_s[:, ot, lt * 512 : (lt + 1) * 512]
                    else:
                        dest = ktp_s[:, ot - NQT,
                                     64 + lt * 512 : 64 + (lt + 1) * 512]
                    nc.scalar.activation(
                        dest, ps[:], AF.Identity, bias=bqk_s[:, ot : ot + 1]
                    )

                # replicate v-bias across partitions: ones[128,1] @ bv[1,512]
                for nt in range(2):
                    ps = psB.tile([128, 512], F32, tag="ps1", bufs=3,
                                  name=f"bvp{nt}")
                    nc.tensor.matmul(
                        ps[:], lhsT=ones1_s[:],
                        rhs=bv_row[:, nt * 512 : (nt + 1) * 512],
                        start=True, stop=True,
                    )
                    nc.scalar.copy(bvrep_s[:, nt * 512 : (nt + 1) * 512], ps[:])

                # Q/K projection for head pair 0; the rest is interleaved
                for ot in (0, NQT):
                    for lt in range(2):
                        emit_qk_proj(ot, lt)

                # V projection -> vpad (seq-major, shifted +64)
                for lt in range(NQT):
                    for nt in range(2):
                        ps = psB.tile([128, 512], F32, tag="ps1", bufs=3,
                                      name=f"vp{lt}_{nt}")
                        for ct in range(NCT):
                            nc.tensor.matmul(
                                ps[:],
                                lhsT=xT_s[:, ct, lt * 128 : (lt + 1) * 128],
                                rhs=wqk_s[:, ct,
                                          2 * C + nt * 512 : 2 * C + (nt + 1) * 512],
                                start=(ct == 0), stop=(ct == NCT - 1),
                            )
                        vtmp = work.tile([128, 512], BF16, tag="vtmp")
                        nc.vector.scalar_tensor_tensor(
                            out=vtmp[:], in0=ps[:], scalar=1.0,
                            in1=bvrep_s[:, nt * 512 : (nt + 1) * 512],
                            op0=ALU.mult, op1=ALU.add,
                        )
                        sl = slice(nt * 512, (nt + 1) * 512)
                        nc.sync.dma_start(vp_s[64:128, lt, sl], vtmp[0:64, :])
                        nc.sync.dma_start(vp_s[0:64, lt + 1, sl], vtmp[64:128, :])

                def kc_geom(kc):
                    q0 = max(kc * 128 - 128, 0)
                    q1 = min(kc * 128 + 128, L)
                    mi = 1 if kc == 0 else (2 if kc == NKC - 1 else 0)
                    return q0, q1, mi

                LAG = 2

                for hp in range(H // 2):
                    projq = []
                    if hp + 1 < H // 2:
                        for ot in (hp + 1, NQT + hp + 1):
                            for lt in range(2):
                                projq.append((ot, lt))
                    ot_ps = psB.tile([128, L], F32, tag="ot", bufs=2,
                                     name=f"ot_ps{hp}")
                    # per-pair rowsums: rows {0,1} for q<512, {32,33} for rest
                    rs_ps = psB.tile([34, 512], F32, tag="rs", bufs=1,
                                     name=f"rs_ps{hp}")
                    nc.tensor.matmul(
                        rs_ps[:], lhsT=zrow_s[:, 0:34], rhs=zrow_s[:],
                        start=True, stop=True, skip_group_check=True,
                    )
                    for bk in range(2):
                        nc.tensor.matmul(
                            ot_ps[:, bk * 512 : (bk + 1) * 512],
                            lhsT=zrow_s[:, 0:128], rhs=zrow_s[:],
                            start=True, stop=True, skip_group_check=True,
                        )

                    pend = []

                    def do_av(st, hp=hp, ot_ps=ot_ps, rs_ps=rs_ps):
                        (hh, hb, kc, pm, q0, q1) = st
                        h = hp * 2 + hh
                        spans = []
                        for b0 in (0, 512):
                            s0, s1 = max(q0, b0), min(q1, b0 + 512)
                            if s0 < s1:
                                spans.append((s0, s1))
                        for s0, s1 in spans:
                            j0, j1 = s0 - q0, s1 - q0
                            nc.tensor.matmul(
                                ot_ps[hb : hb + 64, s0:s1],
                                lhsT=vp_s[:, kc, h * 64 : (h + 1) * 64],
                                rhs=pm[:, j0:j1],
                                start=False, stop=(kc == NKC - 1),
                                skip_group_check=True,
                            )
                        for s0, s1 in spans:
                            j0, j1 = s0 - q0, s1 - q0
                            rb, c0 = (0, 0) if s0 < 512 else (32, 512)
                            nc.tensor.matmul(
                                rs_ps[rb : rb + 2, s0 - c0 : s1 - c0],
                                lhsT=e2c_s[:, hh, :],
                                rhs=pm[:, j0:j1],
                                start=False,
                                stop=(hh == 1 and kc == NKC - 1),
                                skip_group_check=True,
                            )

                    for kc in range(NKC):
                        if projq and kc % 4 == 1:
                            emit_qk_proj(*projq.pop(0))
                        q0, q1, mi = kc_geom(kc)
                        w = q1 - q0
                        s_pair = [None, None]
                        for hh in range(2):
                            hb = hh * 64
                            s_pair[hh] = psB.tile(
                                [128, 256], F32, tag="ps1", bufs=3,
                                name=f"s{hp}_{kc}_{hh}",
                            )
                            nc.tensor.matmul(
                                s_pair[hh][:, 0:w],
                                lhsT=ktp_s[hb : hb + 64, hp,
                                           kc * 128 : (kc + 1) * 128],
                                rhs=qT_s[hb : hb + 64, hp, q0:q1],
                                start=True, stop=True,
                            )
                        for hh in range(2):
                            hb = hh * 64
                            p_sb = work.tile([128, 256], BF16, tag="p", bufs=4)
                            nc.scalar.activation(p_sb[:, 0:w],
                                                 s_pair[hh][:, 0:w],
                                                 AF.Exp, scale=SCALE)
                            pm = work.tile([128, 256], BF16, tag="pm", bufs=6)
                            nc.vector.tensor_mul(pm[:, 0:w], p_sb[:, 0:w],
                                                 masks_s[:, mi, 0:w])
                            pend.append((hh, hb, kc, pm, q0, q1))
                            if len(pend) > LAG:
                                do_av(pend.pop(0))
                    while pend:
                        do_av(pend.pop(0))
                    while projq:
                        emit_qk_proj(*projq.pop(0))

                    # per-pair deferred softmax normalization, overlapped
                    # with the next pair's attention
                    rs_sb = work.tile([34, 512], F32, tag="rssb", bufs=2)
                    nc.scalar.copy(rs_sb[:], rs_ps[:])
                    rcp = work.tile([2, L], F32, tag="rcp", bufs=2)
                    nc.vector.reciprocal(rcp[:, 0:512], rs_sb[0:2, :])
                    nc.vector.reciprocal(rcp[:, 512:L], rs_sb[32:34, :])
                    rcpb = work.tile([2, L], BF16, tag="rcpb", bufs=2)
                    nc.vector.tensor_copy(rcpb[:], rcp[:])
                    for lt in range(2):
                        rep = psB.tile([128, 512], F32, tag="ps1", bufs=3,
                                       name=f"rep{hp}_{lt}")
                        nc.tensor.matmul(
                            rep[:],
                            lhsT=e2r_s[:],
                            rhs=rcpb[:, lt * 512 : (lt + 1) * 512],
                            start=True, stop=True,
                        )
                        rep_sb = work.tile([128, 512], F32, tag="repsb",
                                           bufs=2)
                        nc.scalar.copy(rep_sb[:], rep[:])
                        # fused: evacuate O^T and apply 1/rowsum
                        nc.vector.tensor_mul(
                            oT_s[:, hp, lt * 512 : (lt + 1) * 512],
                            ot_ps[:, lt * 512 : (lt + 1) * 512],
                            rep_sb[:],
                        )

            # ---- phase 4: out projection -> y ----
            with tc.tile_pool(name="psC", bufs=2, space="PSUM") as psC:
                for lt in range(NQT):
                    for mt in range(2):
                        ps = psC.tile([128, 512], F32, tag="proj")
                        for ct in range(NCT):
                            nc.tensor.matmul(
                                ps[:],
                                lhsT=oT_s[:, ct, lt * 128 : (lt + 1) * 128],
                                rhs=wo_s[:, ct, mt * 512 : (mt + 1) * 512],
                                start=(ct == 0), stop=(ct == NCT - 1),
                            )
                        yb = work.tile([128, 512], F32, tag="yb")
                        nc.scalar.copy(yb[:], ps[:])
                        nc.sync.dma_start(
                            y_d[lt * 128 : (lt + 1) * 128,
                                mt * 512 : (mt + 1) * 512],
                            yb[:],
                        )

    nc.compile()
    return nc


def _get_nc():
    if "nc" not in _CACHED:
        _CACHED["nc"] = _build_nc()
    return _CACHED["nc"]


def _prep_in_maps(x, in_proj_w, in_proj_b, out_w):
    bf = ml_dtypes.bfloat16
    wqkT = np.ascontiguousarray(in_proj_w.T).astype(bf)
    woT = np.ascontiguousarray(out_w.T).astype(bf)
    bqk = np.ascontiguousarray(in_proj_b[: 2 * C]).astype(np.float32)
    bv = np.ascontiguousarray(in_proj_b[2 * C :]).astype(np.float32)
    in_maps = []
    for b in range(B):
        xT = np.ascontiguousarray(x[b].T).astype(bf)
        in_maps.append(
            {"xT": xT, "wqkT": wqkT, "woT": woT, "bqk": bqk, "bv": bv}
        )
    return in_maps


def kernel(x, in_proj_w, in_proj_b, out_w, out_b, _trace=False):
    nc = _get_nc()
    in_maps = _prep_in_maps(x, in_proj_w, in_proj_b, out_w)
    res = run_bass_kernel_spmd(nc, in_maps, list(range(NCORES)), trace=_trace)
    _CACHED["last_result"] = res
    y = np.stack([res.results[i]["y"] for i in range(NCORES)], axis=0)
    return (y + out_b[None, None, :].astype(np.float32)).astype(np.float32)


# revision 12
# speedup vs baseline: 1.0690x; 1.0690x over previous
"""Trainium2 Bass kernel for local-window multihead attention.

Problem: B=8, L=1024, C=1024, H=16 heads, head_dim=64, window_size=128
(positions attend to |i-j| <= 64). qkv in-projection + banded softmax
attention + out-projection.

Sharding: data-parallel - one batch element per NeuronCore (8 cores).

Per-core dataflow (bf16 matmuls, fp32 PSUM accumulation):
  xT (host-transposed bf16) --QK-proj--> qT/ktpad  [channel, seq] layout
                            --V-proj --> vpad      [seq, channel] layout
  attention is K-MAJOR: for each (head pair, key-chunk kc) compute
  St = K_chunk^T Q_window for both heads (row-group-packed K=64 matmuls),
  exp on ScalarE, multiplicative band mask on VectorE, then
  O^T += V_chunk^T @ P_t accumulated in PSUM across overlapping chunks,
  rowsums via indicator matmuls into a per-pair PSUM tile.  Per-pair
  softmax normalization (recip + K=2 replication matmul + fused
  multiply-evacuate) overlaps the next pair's attention.  QK-projection
  of pair hp+1 is interleaved into pair hp's stream to keep TensorE
  dense (HAM stays at 2.4 GHz).  out-proj from oT [c, l] -> y [l, c].
"""

import numpy as np
import ml_dtypes

import concourse.bass as bass
import concourse.mybir as mybir
import concourse.tile as tile
from concourse import bacc
from concourse.bass_utils import run_bass_kernel_spmd

BF16 = mybir.dt.bfloat16
F32 = mybir.dt.float32
F32R = mybir.dt.float32r

B, L, C, H, HD = 8, 1024, 1024, 16, 64
WIN = 128
NCORES = 8
NQT = L // 128
NCT = C // 128
NKC = NQT + 1           # key chunks in padded [-64, L+64) key space
SCALE = 1.0 / 8.0       # 1/sqrt(HD)

_CACHED = {}


def _build_nc():
    nc = bacc.Bacc(
        "TRN2", target_bir_lowering=False, debug=False, num_devices=NCORES
    )

    xT_d = nc.dram_tensor("xT", [C, L], BF16, kind="ExternalInput").ap()
    wqk_d = nc.dram_tensor("wqkT", [C, 3 * C], BF16, kind="ExternalInput").ap()
    wo_d = nc.dram_tensor("woT", [C, C], BF16, kind="ExternalInput").ap()
    bqk_d = nc.dram_tensor("bqk", [2 * C], F32, kind="ExternalInput").ap()
    bv_d = nc.dram_tensor("bv", [C], F32, kind="ExternalInput").ap()
    y_d = nc.dram_tensor("y", [L, C], F32, kind="ExternalOutput").ap()

    AF = mybir.ActivationFunctionType
    ALU = mybir.AluOpType

    with tile.TileContext(nc) as tc:
        with (
            tc.tile_pool(name="const", bufs=1) as const,
            tc.tile_pool(name="work", bufs=3) as work,
        ):
            # ---- persistent SBUF tensors ----
            wqk_s = const.tile([128, NCT, 3 * C], BF16, tag="wqk")  # [c_p, ct, o]
            wo_s = const.tile([128, NCT, C], BF16, tag="wo")
            xT_s = const.tile([128, NCT, L], BF16, tag="xT")        # [c_p, ct, l]
            qT_s = const.tile([128, NQT, L], BF16, tag="qT")        # [q-chan, ot, l]
            ktp_s = const.tile([128, NQT, L + 128], BF16, tag="ktp")  # k padded
            vp_s = const.tile([128, NKC, C], BF16, tag="vp")        # v rows +64
            oT_s = const.tile([128, NCT, L], BF16, tag="oT")        # [c_p, ct, l]
            bqk_s = const.tile([128, 2 * C // 128], F32, tag="bqk")
            bv_row = const.tile([1, C], F32, tag="bvrow")
            bvrep_s = const.tile([128, C], F32, tag="bvrep")
            ones1_s = const.tile([1, 128], F32, tag="ones1")
            zrow_s = const.tile([1, 512], BF16, tag="zrow")
            masks_s = const.tile([128, 3, 256], BF16, tag="masks")
            e2c_s = const.tile([128, 2, 2], BF16, tag="e2c")   # rowsum pick
            e2r_s = const.tile([2, 128], BF16, tag="e2r")       # rc replicate

            # ---- input DMAs, staged so phase 1 starts early ----
            nc.sync.dma_start(bqk_s[:], bqk_d.rearrange("(ot p) -> p ot", p=128))
            nc.sync.dma_start(bv_row[:], bv_d.rearrange("(p c) -> p c", p=1))
            nc.sync.dma_start(xT_s[:], xT_d.rearrange("(ct p) l -> p ct l", p=128))
            wqk_r = wqk_d.rearrange("(ct p) o -> p ct o", p=128)
            # q/k columns for early head pairs first, v-part last
            for o0 in (0, 1024, 512, 1536, 2048, 2560):
                nc.sync.dma_start(wqk_s[:, :, o0 : o0 + 512],
                                  wqk_r[:, :, o0 : o0 + 512])
            nc.sync.dma_start(wo_s[:], wo_d.rearrange("(ct p) o -> p ct o", p=128))

            # ---- constants ----
            nc.gpsimd.memset(ones1_s[:], 1.0)
            nc.gpsimd.memset(zrow_s[:], 0.0)

            # band mask (kc interior): valid iff 0 <= jq - p <= 128
            m_band = masks_s[:, 0, :]
            nc.gpsimd.memset(m_band, 1.0)
            nc.gpsimd.affine_select(
                m_band, m_band, compare_op=ALU.is_ge, fill=0.0,
                base=0, pattern=[[1, 256]], channel_multiplier=-1,
            )
            nc.gpsimd.affine_select(
                m_band, m_band, compare_op=ALU.is_ge, fill=0.0,
                base=128, pattern=[[-1, 256]], channel_multiplier=1,
            )
            # kc = 0 (cols [0,128)): valid iff p >= jq and p >= 64
            m_lo = masks_s[:, 1, 0:128]
            nc.gpsimd.memset(m_lo, 1.0)
            nc.gpsimd.affine_select(
                m_lo, m_lo, compare_op=ALU.is_ge, fill=0.0,
                base=0, pattern=[[-1, 128]], channel_multiplier=1,
            )
            nc.gpsimd.memset(masks_s[0:64, 1, 0:128], 0.0)
            # kc = NKC-1 (cols [0,128)): valid iff jq >= p and p < 64
            m_hi = masks_s[:, 2, 0:128]
            nc.gpsimd.memset(m_hi, 1.0)
            nc.gpsimd.affine_select(
                m_hi, m_hi, compare_op=ALU.is_ge, fill=0.0,
                base=0, pattern=[[1, 128]], channel_multiplier=-1,
            )
            nc.gpsimd.memset(masks_s[64:128, 2, 0:128], 0.0)

            # e2c[p, hh, j] = (j == hh); e2r[j, m] = (m // 64 == j)
            nc.gpsimd.memset(e2c_s[:], 1.0)
            nc.gpsimd.affine_select(
                e2c_s[:], e2c_s[:], compare_op=ALU.is_equal, fill=0.0,
                base=0, pattern=[[-1, 2], [1, 2]], channel_multiplier=0,
            )
            nc.gpsimd.memset(e2r_s[:], 1.0)
            nc.gpsimd.affine_select(
                e2r_s[:], e2r_s[:], compare_op=ALU.is_equal, fill=0.0,
                base=0, pattern=[[1, 2], [0, 64]], channel_multiplier=-1,
            )

            # zero padded edges of ktpad / vpad
            for ot in range(NQT):
                nc.gpsimd.memset(ktp_s[:, ot, 0:64], 0.0)
                nc.gpsimd.memset(ktp_s[:, ot, L + 64 : L + 128], 0.0)
            nc.gpsimd.memset(vp_s[0:64, 0, :], 0.0)
            nc.gpsimd.memset(vp_s[64:128, NKC - 1, :], 0.0)

            # ---- phases 1-3: projections + k-major attention ----
            with tc.tile_pool(name="psB", bufs=1, space="PSUM") as psB:

                def emit_qk_proj(ot, lt):
                    ps = psB.tile([128, 512], F32, tag="proj", bufs=2,
                                  name=f"qkp{ot}_{lt}")
                    for ct in range(NCT):
                        nc.tensor.matmul(
                            ps[:],
                            lhsT=wqk_s[:, ct, ot * 128 : (ot + 1) * 128],
                            rhs=xT_s[:, ct, lt * 512 : (lt + 1) * 512],
                            start=(ct == 0), stop=(ct == NCT - 1),
                        )
                    if ot < NQT:
                        dest = qT_s[:, ot, lt * 512 : (lt + 1) * 512]
                    else:
                        dest = ktp_s[:, ot - NQT,
                                     64 + lt * 512 : 64 + (lt + 1) * 512]
                    nc.scalar.activation(
                        dest, ps[:], AF.Identity, bias=bqk_s[:, ot : ot + 1]
                    )

                # replicate v-bias across partitions: ones[128,1] @ bv[1,512]
                for nt in range(2):
                    ps = psB.tile([128, 512], F32, tag="proj", bufs=2,
                                  name=f"bvp{nt}")
                    nc.tensor.matmul(
                        ps[:], lhsT=ones1_s[:],
                        rhs=bv_row[:, nt * 512 : (nt + 1) * 512],
                        start=True, stop=True,
                    )
                    nc.scalar.copy(bvrep_s[:, nt * 512 : (nt + 1) * 512], ps[:])

                # Q/K projection for head pair 0; the rest is interleaved
                for ot in (0, NQT):
                    for lt in range(2):
                        emit_qk_proj(ot, lt)

                # V projection -> vpad (seq-major, shifted +64)
                for lt in range(NQT):
                    for nt in range(2):
                        ps = psB.tile([128, 512], F32, tag="proj", bufs=2,
                                      name=f"vp{lt}_{nt}")
                        for ct in range(NCT):
                            nc.tensor.matmul(
                                ps[:],
                                lhsT=xT_s[:, ct, lt * 128 : (lt + 1) * 128],
                                rhs=wqk_s[:, ct,
                                          2 * C + nt * 512 : 2 * C + (nt + 1) * 512],
                                start=(ct == 0), stop=(ct == NCT - 1),
                            )
                        vtmp = work.tile([128, 512], BF16, tag="vtmp")
                        nc.vector.scalar_tensor_tensor(
                            out=vtmp[:], in0=ps[:], scalar=1.0,
                            in1=bvrep_s[:, nt * 512 : (nt + 1) * 512],
                            op0=ALU.mult, op1=ALU.add,
                        )
                        sl = slice(nt * 512, (nt + 1) * 512)
                        nc.sync.dma_start(vp_s[64:128, lt, sl], vtmp[0:64, :])
                        nc.sync.dma_start(vp_s[0:64, lt + 1, sl], vtmp[64:128, :])

                def kc_geom(kc):
                    q0 = max(kc * 128 - 128, 0)
                    q1 = min(kc * 128 + 128, L)
                    mi = 1 if kc == 0 else (2 if kc == NKC - 1 else 0)
                    return q0, q1, mi

                LAG = 3

                for hp in range(H // 2):
                    projq = []
                    if hp + 1 < H // 2:
                        for ot in (hp + 1, NQT + hp + 1):
                            for lt in range(2):
                                projq.append((ot, lt))
                    ot_ps = psB.tile([128, L], F32, tag="ot", bufs=1,
                                     name=f"ot_ps{hp}")
                    # per-pair rowsums: rows {0,1} for q<512, {32,33} for rest
                    rs_ps = psB.tile([34, 512], F32, tag="rs", bufs=1,
                                     name=f"rs_ps{hp}")
                    nc.tensor.matmul(
                        rs_ps[:], lhsT=zrow_s[:, 0:34], rhs=zrow_s[:],
                        start=True, stop=True, skip_group_check=True,
                    )
                    for bk in range(2):
                        nc.tensor.matmul(
                            ot_ps[:, bk * 512 : (bk + 1) * 512],
                            lhsT=zrow_s[:, 0:128], rhs=zrow_s[:],
                            start=True, stop=True, skip_group_check=True,
                        )

                    pend = []

                    def do_av(st, hp=hp, ot_ps=ot_ps, rs_ps=rs_ps):
                        (hh, hb, kc, pm, q0, q1) = st
                        h = hp * 2 + hh
                        spans = []
                        for b0 in (0, 512):
                            s0, s1 = max(q0, b0), min(q1, b0 + 512)
                            if s0 < s1:
                                spans.append((s0, s1))
                        for s0, s1 in spans:
                            j0, j1 = s0 - q0, s1 - q0
                            nc.tensor.matmul(
                                ot_ps[hb : hb + 64, s0:s1],
                                lhsT=vp_s[:, kc, h * 64 : (h + 1) * 64],
                                rhs=pm[:, j0:j1],
                                start=False, stop=(kc == NKC - 1),
                                skip_group_check=True,
                            )
                        for s0, s1 in spans:
                            j0, j1 = s0 - q0, s1 - q0
                            rb, c0 = (0, 0) if s0 < 512 else (32, 512)
                            nc.tensor.matmul(
                                rs_ps[rb : rb + 2, s0 - c0 : s1 - c0],
                                lhsT=e2c_s[:, hh, :],
                                rhs=pm[:, j0:j1],
                                start=False,
                                stop=(hh == 1 and kc == NKC - 1),
                                skip_group_check=True,
                            )

                    for kc in range(NKC):
                        if projq and kc % 4 == 1:
                            emit_qk_proj(*projq.pop(0))
                        q0, q1, mi = kc_geom(kc)
                        w = q1 - q0
                        s_pair = [None, None]
                        for hh in range(2):
                            hb = hh * 64
                            s_pair[hh] = psB.tile(
                                [128, 256], F32, tag="s", bufs=3,
                                name=f"s{hp}_{kc}_{hh}",
                            )
                            nc.tensor.matmul(
                                s_pair[hh][:, 0:w],
                                lhsT=ktp_s[hb : hb + 64, hp,
                                           kc * 128 : (kc + 1) * 128],
                                rhs=qT_s[hb : hb + 64, hp, q0:q1],
                                start=True, stop=True,
                            )
                        for hh in range(2):
                            hb = hh * 64
                            p_sb = work.tile([128, 256], BF16, tag="p", bufs=4)
                            nc.scalar.activation(p_sb[:, 0:w],
                                                 s_pair[hh][:, 0:w],
                                                 AF.Exp, scale=SCALE)
                            pm = work.tile([128, 256], BF16, tag="pm", bufs=6)
                            nc.vector.tensor_mul(pm[:, 0:w], p_sb[:, 0:w],
                                                 masks_s[:, mi, 0:w])
                            pend.append((hh, hb, kc, pm, q0, q1))
                            if len(pend) > LAG:
                                do_av(pend.pop(0))
                    while pend:
                        do_av(pend.pop(0))
                    while projq:
                        emit_qk_proj(*projq.pop(0))

                    # per-pair deferred softmax normalization, overlapped
                    # with the next pair's attention
                    rs_sb = work.tile([34, 512], F32, tag="rssb", bufs=2)
                    nc.scalar.copy(rs_sb[:], rs_ps[:])
                    rcp = work.tile([2, L], F32, tag="rcp", bufs=2)
                    nc.vector.reciprocal(rcp[:, 0:512], rs_sb[0:2, :])
                    nc.vector.reciprocal(rcp[:, 512:L], rs_sb[32:34, :])
                    rcpb = work.tile([2, L], BF16, tag="rcpb", bufs=2)
                    nc.vector.tensor_copy(rcpb[:], rcp[:])
                    for lt in range(2):
                        rep = psB.tile([128, 512], F32, tag="proj", bufs=2,
                                       name=f"rep{hp}_{lt}")
                        nc.tensor.matmul(
                            rep[:],
                            lhsT=e2r_s[:],
                            rhs=rcpb[:, lt * 512 : (lt + 1) * 512],
                            start=True, stop=True,
                        )
                        rep_sb = work.tile([128, 512], F32, tag="repsb",
                                           bufs=2)
                        nc.scalar.copy(rep_sb[:], rep[:])
                        # fused: evacuate O^T and apply 1/rowsum
                        nc.vector.tensor_mul(
                            oT_s[:, hp, lt * 512 : (lt + 1) * 512],
                            ot_ps[:, lt * 512 : (lt + 1) * 512],
                            rep_sb[:],
                        )

            # ---- phase 4: out projection -> y ----
            with tc.tile_pool(name="psC", bufs=2, space="PSUM") as psC:
                for lt in range(NQT):
                    for mt in range(2):
                        ps = psC.tile([128, 512], F32, tag="proj")
                        for ct in range(NCT):
                            nc.tensor.matmul(
                                ps[:],
                                lhsT=oT_s[:, ct, lt * 128 : (lt + 1) * 128],
                                rhs=wo_s[:, ct, mt * 512 : (mt + 1) * 512],
                                start=(ct == 0), stop=(ct == NCT - 1),
                            )
                        yb = work.tile([128, 512], F32, tag="yb")
                        nc.scalar.copy(yb[:], ps[:])
                        nc.sync.dma_start(
                            y_d[lt * 128 : (lt + 1) * 128,
                                mt * 512 : (mt + 1) * 512],
                            yb[:],
                        )

    nc.compile()
    return nc


def _get_nc():
    if "nc" not in _CACHED:
        _CACHED["nc"] = _build_nc()
    return _CACHED["nc"]


def _prep_in_maps(x, in_proj_w, in_proj_b, out_w):
    bf = ml_dtypes.bfloat16
    wqkT = np.ascontiguousarray(in_proj_w.T).astype(bf)
    woT = np.ascontiguousarray(out_w.T).astype(bf)
    bqk = np.ascontiguousarray(in_proj_b[: 2 * C]).astype(np.float32)
    bv = np.ascontiguousarray(in_proj_b[2 * C :]).astype(np.float32)
    in_maps = []
    for b in range(B):
        xT = np.ascontiguousarray(x[b].T).astype(bf)
        in_maps.append(
            {"xT": xT, "wqkT": wqkT, "woT": woT, "bqk": bqk, "bv": bv}
        )
    return in_maps


def kernel(x, in_proj_w, in_proj_b, out_w, out_b, _trace=False):
    nc = _get_nc()
    in_maps = _prep_in_maps(x, in_proj_w, in_proj_b, out_w)
    res = run_bass_kernel_spmd(nc, in_maps, list(range(NCORES)), trace=_trace)
    _CACHED["last_result"] = res
    y = np.stack([res.results[i]["y"] for i in range(NCORES)], axis=0)
    return (y + out_b[None, None, :].astype(np.float32)).astype(np.float32)
